# revision 17
# baseline (speedup 1.0000x reference)
"""Trainium2 Bass kernel for nn_CNNAttention (cosine-normalized linear attention).

Math: the reference "attention" has no softmax -- after folding the cosine
normalization into q^ = q/sqrt(|q|^2+eps), k^ = k/sqrt(|k|^2+eps) (the +eps
on the attn denominator is a ~2e-6 relative perturbation, far below fp32
matmul noise), the computation is linear and reassociates:

    o = (q^T k^  / norm) v   ==   o[d,n] = sum_e S[e,d] q^[e,n],
    S = sum_m k^T[m,e] v^T[m,d]            (64x64 per head)

so no 4096x4096 attention matrix is ever materialized.

Sharding: each of the 8 cores owns 512 of the 4096 spatial positions, both
as its q/o/y slice and as its k/v slice for the S partial contraction.
Cross-core comms: one 256KB AllReduce for S, one 2KB AllReduce for BN stats.
"""

import numpy as np

HEADS = 8
D = 64
C = 256
N = 4096
NCORES = 8
SL = N // NCORES      # 512 spatial positions per core
INNER = HEADS * D     # 512
SMOOTH = 1e-4
BN_EPS = 1e-5
P = 128

# tunables
MM_DTYPE = "float32r"   # "float32" (4 cyc/row, exact) or "float32r" (1 cyc/row)
DUMMY_WARMUP_AR = False  # absorb first-collective setup cost concurrently w/ compute

_CACHE = {}


def _make_tile_context_cls(tile, mybir, ScopedClock):
    class SplitWaitTileContext(tile.TileContext):
        """Workaround: this walrus build accepts only ONE sync-wait command
        per instruction. Tile attaches one wait per cross-proc dependency.
        After scheduling, splice all-but-one wait of each multi-wait
        instruction onto single-wait NOPs inserted just before it on the
        same engine (engine streams execute in order, so semantics are
        preserved)."""

        def __exit__(self, exc_type, exc_val, exc_tb):
            r = super().__exit__(exc_type, exc_val, exc_tb)
            if exc_type is None:
                self._split_multi_waits()
            return r

        def _split_multi_waits(self):
            nc = self.nc
            k = 0
            for bass_bb in nc.main_func.blocks:
                insts = bass_bb.instructions
                i = 0
                while i < len(insts):
                    inst = insts[i]
                    si = inst.sync_info
                    waits = list(si.on_wait) if si and si.on_wait else []
                    if len(waits) > 1:
                        si.on_wait = [waits[-1]]
                        for w in waits[:-1]:
                            nop = mybir.InstNoOp(
                                name=f"I-waitsplit-{k}", ins=[], outs=[],
                                text_hint="waitsplit", bass_nofuse=True,
                            )
                            nop.engine = inst.engine
                            nop.sync_info = mybir.SyncInfo(
                                on_wait=[w], on_update=[])
                            insts.insert(i, nop)
                            i += 1
                            k += 1
                    i += 1

    return SplitWaitTileContext


def build_nc(mm_dtype=MM_DTYPE, dummy_ar=DUMMY_WARMUP_AR, debug_out=False):
    import concourse.bass as bass
    import concourse.mybir as mybir
    import concourse.tile as tile
    from concourse.vector_clock import ScopedClock

    SplitWaitTileContext = _make_tile_context_cls(tile, mybir, ScopedClock)

    f32 = mybir.dt.float32
    mmdt = getattr(mybir.dt, mm_dtype)
    AF = mybir.ActivationFunctionType
    ALU = mybir.AluOpType
    RG = [list(range(NCORES))]

    def mm(ap):
        return ap

    def bc(ap):
        # bitcast a DRAM f32 AP to the matmul dtype (raw 4-byte copy)
        return ap if mmdt == f32 else ap.bitcast(mmdt)

    nc = bass.Bass("TRN2", target_bir_lowering=False, debug=False,
                   num_devices=NCORES)

    cvec = nc.dram_tensor("cvec", [P, 3], f32, kind="ExternalInput").ap()
    xs = nc.dram_tensor("xs", [C, SL], f32, kind="ExternalInput").ap()
    wq = nc.dram_tensor("wq", [C, INNER], f32, kind="ExternalInput").ap()
    wk = nc.dram_tensor("wk", [C, INNER], f32, kind="ExternalInput").ap()
    wv = nc.dram_tensor("wv", [C, INNER], f32, kind="ExternalInput").ap()
    wo = nc.dram_tensor("wo", [INNER, C], f32, kind="ExternalInput").ap()
    ind = nc.dram_tensor("ind", [P, P], f32, kind="ExternalInput").ap()
    gamma2 = nc.dram_tensor("gamma2", [P, 2], f32, kind="ExternalInput").ap()
    beta2 = nc.dram_tensor("beta2", [P, 2], f32, kind="ExternalInput").ap()
    y = nc.dram_tensor("y", [C, SL], f32, kind="ExternalOutput").ap()
    dbg = {}
    if debug_out:
        for nm, shp in [("d_khat", [P, 4, HEADS, D]), ("d_vt", [P, 4, INNER]),
                        ("d_qhat", [P, 4, SL]), ("d_spart", [64, HEADS, D]),
                        ("d_sfull", [P, HEADS, D]), ("d_o", [P, 4, SL]),
                        ("d_ys", [P, 2, SL]), ("d_stats", [P, 2, 2])]:
            dbg[nm] = nc.dram_tensor(nm, shp, f32, kind="ExternalOutput").ap()

    s_part = nc.dram_tensor("s_part", [64, 512], f32).ap()
    s_full = nc.dram_tensor("s_full", [64, 512], f32, addr_space="Shared").ap()
    st_part = nc.dram_tensor("st_part", [P, 4], f32).ap()
    st_full = nc.dram_tensor("st_full", [NCORES, P, 4], f32, addr_space="Shared").ap()
    if dummy_ar:
        dum_in = nc.dram_tensor("dum_in", [1, 128], f32).ap()
        dum_out = nc.dram_tensor("dum_out", [1, 128], f32, addr_space="Shared").ap()

    with SplitWaitTileContext(nc) as tc:
        with tc.tile_pool(name="persist", bufs=1) as pp, \
             tc.tile_pool(name="scratch", bufs=3) as sp, \
             tc.tile_pool(name="psS", bufs=1, space="PSUM") as psS:

            if dummy_ar:
                # fire a no-dependency collective immediately: absorbs the
                # per-execution collective setup / core-alignment cost while
                # the compute engines run the kv/q stages.
                nc.gpsimd.collective_compute(
                    "AllReduce", ALU.add, replica_groups=RG,
                    ins=[dum_in], outs=[dum_out])

            # ---- persistent SBUF loads ----
            cvec_sb = pp.tile([P, 3], f32, tag="cvec_sb")
            nc.sync.dma_start(cvec_sb[:], cvec)
            for ci, cval in enumerate((SMOOTH, BN_EPS, 1.0 / N)):
                nc.const_aps.aps[(f32, cval)] = cvec_sb[:, ci:ci + 1]
            xs_sb = pp.tile([P, 2, SL], mmdt, tag="xs_sb")
            nc.sync.dma_start(xs_sb[:], bc(xs.rearrange("(kt p) n -> p kt n", p=P)))
            wq_sb = pp.tile([P, 2, INNER], mmdt, tag="wq_sb")
            nc.gpsimd.dma_start(wq_sb[:], bc(wq.rearrange("(kt p) e -> p kt e", p=P)))
            wk_sb = pp.tile([P, 2, INNER], mmdt, tag="wk_sb")
            nc.scalar.dma_start(wk_sb[:], bc(wk.rearrange("(kt p) e -> p kt e", p=P)))
            wv_sb = pp.tile([P, 2, INNER], mmdt, tag="wv_sb")
            nc.gpsimd.dma_start(wv_sb[:], bc(wv.rearrange("(kt p) e -> p kt e", p=P)))
            wo_sb = pp.tile([P, 4, C], mmdt, tag="wo_sb")
            nc.gpsimd.dma_start(wo_sb[:], bc(wo.rearrange("(kt p) c -> p kt c", p=P)))
            ind_sb = pp.tile([P, P], mmdt, tag="ind_sb")
            nc.gpsimd.dma_start(ind_sb[:], bc(ind))
            g2_sb = pp.tile([P, 2], f32, tag="g2_sb")
            nc.gpsimd.dma_start(g2_sb[:], gamma2)
            b2_sb = pp.tile([P, 2], f32, tag="b2_sb")
            nc.gpsimd.dma_start(b2_sb[:], beta2)

            khat = pp.tile([P, 4, HEADS, D], mmdt, tag="khat")
            vt_sb = pp.tile([P, 4, INNER], mmdt, tag="vt_sb")
            qhat = pp.tile([P, 4, SL], mmdt, tag="qhat")
            o_sb = pp.tile([P, 4, SL], mmdt, tag="o_sb")
            s_sb = pp.tile([64, HEADS, D], f32, tag="s_sb")
            sfull_sb = pp.tile([P, HEADS, D], mmdt, tag="sfull_sb")
            ys_sb = pp.tile([P, 2, SL], f32, tag="ys_sb")
            yo_sb = pp.tile([P, 2, SL], f32, tag="yo_sb")
            stats_sb = pp.tile([P, 2, 2], f32, tag="stats_sb")
            stf_sb = pp.tile([P, 2, 2], f32, tag="stf_sb")

            # ---- stage KV + S partials (m = this core's 512 positions) ----
            # one PSUM bank per 128-col S region (matmul dst must be
            # bank-aligned; column-offset dst regions corrupt the bank)
            psum_S = [psS.tile([P, P], f32, tag=f"psum_S{pc}", name=f"psum_S{pc}")
                      for pc in range(4)]
            with tc.tile_pool(name="pskv", bufs=2, space="PSUM") as pskv:
                for mc in range(4):
                    xsl = xs_sb[:, :, mc * P:(mc + 1) * P]
                    psum_kT = pskv.tile([P, INNER], f32, tag="psum_kT")
                    psum_vT = pskv.tile([P, INNER], f32, tag="psum_vT")
                    for kt in range(2):
                        nc.tensor.matmul(psum_kT[:], mm(xsl[:, kt]), mm(wk_sb[:, kt]),
                                         start=(kt == 0), stop=(kt == 1))
                    for kt in range(2):
                        nc.tensor.matmul(psum_vT[:], mm(xsl[:, kt]), mm(wv_sb[:, kt]),
                                         start=(kt == 0), stop=(kt == 1))
                    # row norms of kT per head -> normalize
                    k2 = sp.tile([P, HEADS, D], f32, tag="k2")
                    nc.scalar.activation(k2[:], psum_kT.rearrange("p (g d) -> p g d", g=HEADS),
                                         AF.Square)
                    kn2 = sp.tile([P, HEADS], f32, tag="kn2")
                    nc.vector.tensor_reduce(kn2[:], k2[:], axis=mybir.AxisListType.X,
                                            op=ALU.add)
                    kn = sp.tile([P, HEADS], f32, tag="kn")
                    nc.scalar.activation(kn[:], kn2[:], AF.Sqrt, bias=SMOOTH)
                    kninv = sp.tile([P, HEADS], f32, tag="kninv")
                    nc.vector.reciprocal(kninv[:], kn[:])
                    nc.vector.tensor_tensor(
                        khat[:, mc], psum_kT.rearrange("p (g d) -> p g d", g=HEADS),
                        kninv[:, :, None].to_broadcast((P, HEADS, D)), ALU.mult)
                    nc.scalar.activation(vt_sb[:, mc], psum_vT[:], AF.Copy)
                    # S partial accumulation: 4 column regions of one bank
                    kh_flat = khat[:, mc].rearrange("p g d -> p (g d)")
                    for pc in range(4):
                        nc.tensor.matmul(
                            psum_S[pc][:],
                            mm(kh_flat[:, pc * P:(pc + 1) * P]),
                            mm(vt_sb[:, mc, pc * P:(pc + 1) * P]),
                            start=(mc == 0), stop=(mc == 3))

            for pc in range(4):
                nc.vector.tensor_copy(s_sb[:, 2 * pc, :], psum_S[pc][0:64, 0:64])
                nc.vector.tensor_copy(s_sb[:, 2 * pc + 1, :],
                                      psum_S[pc][64:128, 64:128])
            nc.sync.dma_start(s_part, s_sb.rearrange("p g d -> p (g d)"))
            if debug_out:
                nc.sync.dma_start(dbg["d_khat"], khat.bitcast(f32))
                nc.sync.dma_start(dbg["d_vt"], vt_sb.bitcast(f32))
                nc.sync.dma_start(dbg["d_spart"], s_sb[:])
            nc.gpsimd.collective_compute(
                "AllReduce", ALU.add, replica_groups=RG,
                ins=[s_part], outs=[s_full])
            sf_r = s_full.rearrange("p (g d) -> p g d", g=HEADS)
            nc.sync.dma_start(sfull_sb[0:64], bc(sf_r))
            nc.sync.dma_start(sfull_sb[64:128], bc(sf_r))
            if debug_out:
                nc.sync.dma_start(dbg["d_sfull"], sfull_sb.bitcast(f32))

            # ---- stage Q (n = this core's 512 positions) ----
            with tc.tile_pool(name="psq", bufs=2, space="PSUM") as psq:
                for pc in range(4):
                    psum_q = psq.tile([P, SL], f32, tag="psum_q")
                    for kt in range(2):
                        nc.tensor.matmul(psum_q[:],
                                         mm(wq_sb[:, kt, pc * P:(pc + 1) * P]),
                                         mm(xs_sb[:, kt]),
                                         start=(kt == 0), stop=(kt == 1))
                    q2 = sp.tile([P, SL], mmdt, tag="q2")
                    nc.scalar.activation(q2[:], psum_q[:], AF.Square)
                    psum_nrm = psq.tile([P, SL], f32, tag="psum_nrm")
                    nc.tensor.matmul(psum_nrm[:], mm(ind_sb[:]), mm(q2[:]),
                                     start=True, stop=True)
                    qn = sp.tile([P, SL], f32, tag="qn")
                    nc.scalar.activation(qn[:], psum_nrm[:], AF.Sqrt, bias=SMOOTH)
                    qninv = sp.tile([P, SL], f32, tag="qninv")
                    nc.vector.reciprocal(qninv[:], qn[:])
                    nc.vector.tensor_tensor(qhat[:, pc], psum_q[:], qninv[:], ALU.mult)

            # ---- stage O: o[e,n] = sum_e' S[e',e] qhat[e',n], 2 heads/chunk ----
            with tc.tile_pool(name="pso", bufs=2, space="PSUM") as pso:
                for pc in range(4):
                    # two heads per chunk; PE row-groups 0-1 and 2-3 run
                    # these K=64 matmuls concurrently (row packing)
                    psum_oa = pso.tile([64, SL], f32, tag="psum_oa")
                    psum_ob = pso.tile([64, SL], f32, tag="psum_ob")
                    nc.tensor.matmul(psum_oa[:],
                                     mm(sfull_sb[0:64, 2 * pc, :]),
                                     mm(qhat[0:64, pc]),
                                     start=True, stop=True)
                    nc.tensor.matmul(psum_ob[:],
                                     mm(sfull_sb[64:128, 2 * pc + 1, :]),
                                     mm(qhat[64:128, pc]),
                                     start=True, stop=True)
                    nc.scalar.activation(o_sb[0:64, pc], psum_oa[:], AF.Copy)
                    nc.vector.tensor_copy(o_sb[64:128, pc], psum_ob[:])

            # ---- stage Y: y = w_out @ o, BN stats partials ----
            with tc.tile_pool(name="psy", bufs=2, space="PSUM") as psy:
                for cc in range(2):
                    psum_y = psy.tile([P, SL], f32, tag="psum_y")
                    for kt in range(4):
                        nc.tensor.matmul(psum_y[:],
                                         mm(wo_sb[:, kt, cc * P:(cc + 1) * P]),
                                         mm(o_sb[:, kt]),
                                         start=(kt == 0), stop=(kt == 3))
                    nc.scalar.activation(ys_sb[:, cc], psum_y[:], AF.Copy,
                                         accum_out=stats_sb[:, cc, 0:1])
                    junk = sp.tile([P, SL], f32, tag="junk")
                    nc.scalar.activation(junk[:], psum_y[:], AF.Square,
                                         accum_out=stats_sb[:, cc, 1:2])

            if debug_out:
                nc.sync.dma_start(dbg["d_qhat"], qhat.bitcast(f32))
                nc.sync.dma_start(dbg["d_o"], o_sb.bitcast(f32))
                nc.sync.dma_start(dbg["d_ys"], ys_sb[:])
                nc.sync.dma_start(dbg["d_stats"], stats_sb[:])
            nc.sync.dma_start(st_part, stats_sb.rearrange("p a b -> p (a b)"))
            # AllGather (7 ring steps) + local reduce beats AllReduce (14 steps)
            nc.gpsimd.collective_compute(
                "AllGather", ALU.bypass, replica_groups=RG,
                ins=[st_part], outs=[st_full])
            stf8 = pp.tile([P, 4, NCORES], f32, tag="stf8")
            nc.sync.dma_start(stf8[:], st_full.rearrange("g p s -> p s g"))
            nc.vector.tensor_reduce(stf_sb.rearrange("p a b -> p (a b)"), stf8[:],
                                    axis=mybir.AxisListType.X, op=ALU.add)

            # ---- BN finalize: scale/shift per channel ----
            me = pp.tile([P, 2, 2], f32, tag="me")
            nc.scalar.activation(me[:], stf_sb[:], AF.Copy, scale=1.0 / N)
            mean = me[:, :, 0]
            ex2 = me[:, :, 1]
            var = pp.tile([P, 2], f32, tag="var")
            nc.vector.tensor_tensor(var[:], mean, mean, ALU.mult)
            nc.vector.tensor_tensor(var[:], ex2, var[:], ALU.subtract)
            std = pp.tile([P, 2], f32, tag="std")
            nc.scalar.activation(std[:], var[:], AF.Sqrt, bias=BN_EPS)
            rstd = pp.tile([P, 2], f32, tag="rstd")
            nc.vector.reciprocal(rstd[:], std[:])
            scale = pp.tile([P, 2], f32, tag="scale")
            nc.vector.tensor_tensor(scale[:], g2_sb[:], rstd[:], ALU.mult)
            shift = pp.tile([P, 2], f32, tag="shift")
            nc.vector.tensor_tensor(shift[:], mean[:], scale[:], ALU.mult)
            nc.vector.tensor_tensor(shift[:], b2_sb[:], shift[:], ALU.subtract)

            # ---- apply BN + ReLU, store ----
            y_r = y.rearrange("(cc p) n -> p cc n", p=P)
            for cc in range(2):
                nc.scalar.activation(yo_sb[:, cc], ys_sb[:, cc], AF.Relu,
                                     bias=shift[:, cc:cc + 1],
                                     scale=scale[:, cc:cc + 1])
                nc.sync.dma_start(y_r[:, cc], yo_sb[:, cc])

    return nc


def _prep_inputs(x, w_qkv, w_out, gamma, beta):
    X = np.ascontiguousarray(x.reshape(C, N))
    wq = np.ascontiguousarray(w_qkv[0:INNER].T)
    wk = np.ascontiguousarray(w_qkv[INNER:2 * INNER].T)
    wv = np.ascontiguousarray(w_qkv[2 * INNER:3 * INNER].T)
    wo = np.ascontiguousarray(w_out.T)
    r = np.arange(P)
    ind = (((r[:, None] < D) == (r[None, :] < D))).astype(np.float32)
    ind = np.ascontiguousarray(ind)
    gamma2 = np.ascontiguousarray(gamma.reshape(2, P).T)
    beta2 = np.ascontiguousarray(beta.reshape(2, P).T)
    cvec = np.tile(np.array([SMOOTH, BN_EPS, 1.0 / N], np.float32), (P, 1))
    common = dict(wq=wq, wk=wk, wv=wv, wo=wo, ind=ind, gamma2=gamma2,
                  beta2=beta2, cvec=np.ascontiguousarray(cvec))
    in_maps = []
    for i in range(NCORES):
        m = dict(common)
        m["xs"] = np.ascontiguousarray(X[:, i * SL:(i + 1) * SL])
        in_maps.append(m)
    return in_maps


def kernel(x, w_qkv, w_out, gamma, beta):
    from concourse.bass_utils import run_bass_kernel_spmd

    if "nc" not in _CACHE:
        _CACHE["nc"] = build_nc()
    nc = _CACHE["nc"]

    in_maps = _prep_inputs(
        np.asarray(x, dtype=np.float32),
        np.asarray(w_qkv, dtype=np.float32),
        np.asarray(w_out, dtype=np.float32),
        np.asarray(gamma, dtype=np.float32),
        np.asarray(beta, dtype=np.float32),
    )
    res = run_bass_kernel_spmd(nc, in_maps, list(range(NCORES)))
    out = np.concatenate([res.results[i]["y"] for i in range(NCORES)], axis=1)
    return out.reshape(1, C, 64, 64).astype(np.float32)


def run_traced(x, w_qkv, w_out, gamma, beta, **kw):
    """Like kernel() but with NTFF tracing; returns (output, BassKernelResults)."""
    from concourse.bass_utils import run_bass_kernel_spmd

    if "nc" not in _CACHE:
        _CACHE["nc"] = build_nc()
    nc = _CACHE["nc"]
    in_maps = _prep_inputs(np.asarray(x, np.float32), np.asarray(w_qkv, np.float32),
                           np.asarray(w_out, np.float32), np.asarray(gamma, np.float32),
                           np.asarray(beta, np.float32))
    res = run_bass_kernel_spmd(nc, in_maps, list(range(NCORES)), trace=True, **kw)
    out = np.concatenate([res.results[i]["y"] for i in range(NCORES)], axis=1)
    return out.reshape(1, C, 64, 64).astype(np.float32), res


# revision 18
# speedup vs baseline: 1.0730x; 1.0730x over previous
"""Trainium2 Bass kernel for nn_CNNAttention (cosine-normalized linear attention).

Math: the reference "attention" has no softmax -- after folding the cosine
normalization into q^ = q/sqrt(|q|^2+eps), k^ = k/sqrt(|k|^2+eps) (the +eps
on the attn denominator is a ~2e-6 relative perturbation, far below fp32
matmul noise), the computation is linear and reassociates:

    o = (q^T k^  / norm) v   ==   o[d,n] = sum_e S[e,d] q^[e,n],
    S = sum_m k^T[m,e] v^T[m,d]            (64x64 per head)

so no 4096x4096 attention matrix is ever materialized.

Sharding: each of the 8 cores owns 512 of the 4096 spatial positions, both
as its q/o/y slice and as its k/v slice for the S partial contraction.
Cross-core comms: one 256KB AllReduce for S, one 2KB AllReduce for BN stats.
"""

import numpy as np

HEADS = 8
D = 64
C = 256
N = 4096
NCORES = 8
SL = N // NCORES      # 512 spatial positions per core
INNER = HEADS * D     # 512
SMOOTH = 1e-4
BN_EPS = 1e-5
P = 128

# tunables
MM_DTYPE = "float32r"   # "float32" (4 cyc/row, exact) or "float32r" (1 cyc/row)
DUMMY_WARMUP_AR = False  # absorb first-collective setup cost concurrently w/ compute

_CACHE = {}


def _make_tile_context_cls(tile, mybir, ScopedClock):
    class SplitWaitTileContext(tile.TileContext):
        """Workaround: this walrus build accepts only ONE sync-wait command
        per instruction. Tile attaches one wait per cross-proc dependency.
        After scheduling, splice all-but-one wait of each multi-wait
        instruction onto single-wait NOPs inserted just before it on the
        same engine (engine streams execute in order, so semantics are
        preserved)."""

        def __exit__(self, exc_type, exc_val, exc_tb):
            r = super().__exit__(exc_type, exc_val, exc_tb)
            if exc_type is None:
                self._split_multi_waits()
            return r

        def _split_multi_waits(self):
            nc = self.nc
            k = 0
            for bass_bb in nc.main_func.blocks:
                insts = bass_bb.instructions
                i = 0
                while i < len(insts):
                    inst = insts[i]
                    si = inst.sync_info
                    waits = list(si.on_wait) if si and si.on_wait else []
                    if len(waits) > 1:
                        si.on_wait = [waits[-1]]
                        for w in waits[:-1]:
                            nop = mybir.InstNoOp(
                                name=f"I-waitsplit-{k}", ins=[], outs=[],
                                text_hint="waitsplit", bass_nofuse=True,
                            )
                            nop.engine = inst.engine
                            nop.sync_info = mybir.SyncInfo(
                                on_wait=[w], on_update=[])
                            insts.insert(i, nop)
                            i += 1
                            k += 1
                    i += 1

    return SplitWaitTileContext


def build_nc(mm_dtype=MM_DTYPE, dummy_ar=DUMMY_WARMUP_AR, debug_out=False):
    import concourse.bass as bass
    import concourse.mybir as mybir
    import concourse.tile as tile
    from concourse.vector_clock import ScopedClock

    SplitWaitTileContext = _make_tile_context_cls(tile, mybir, ScopedClock)

    f32 = mybir.dt.float32
    mmdt = getattr(mybir.dt, mm_dtype)
    AF = mybir.ActivationFunctionType
    ALU = mybir.AluOpType
    RG = [list(range(NCORES))]

    def mm(ap):
        return ap

    def bc(ap):
        # bitcast a DRAM f32 AP to the matmul dtype (raw 4-byte copy)
        return ap if mmdt == f32 else ap.bitcast(mmdt)

    nc = bass.Bass("TRN2", target_bir_lowering=False, debug=False,
                   num_devices=NCORES)

    cvec = nc.dram_tensor("cvec", [P, 3], f32, kind="ExternalInput").ap()
    xs = nc.dram_tensor("xs", [C, SL], f32, kind="ExternalInput").ap()
    wq = nc.dram_tensor("wq", [C, INNER], f32, kind="ExternalInput").ap()
    wk = nc.dram_tensor("wk", [C, INNER], f32, kind="ExternalInput").ap()
    wv = nc.dram_tensor("wv", [C, INNER], f32, kind="ExternalInput").ap()
    wo = nc.dram_tensor("wo", [INNER, C], f32, kind="ExternalInput").ap()
    ind = nc.dram_tensor("ind", [P, P], f32, kind="ExternalInput").ap()
    gamma2 = nc.dram_tensor("gamma2", [P, 2], f32, kind="ExternalInput").ap()
    beta2 = nc.dram_tensor("beta2", [P, 2], f32, kind="ExternalInput").ap()
    y = nc.dram_tensor("y", [C, SL], f32, kind="ExternalOutput").ap()
    dbg = {}
    if debug_out:
        for nm, shp in [("d_khat", [P, 4, HEADS, D]), ("d_vt", [P, 4, INNER]),
                        ("d_qhat", [P, 4, SL]), ("d_spart", [64, HEADS, D]),
                        ("d_sfull", [P, HEADS, D]), ("d_o", [P, 4, SL]),
                        ("d_ys", [P, 2, SL]), ("d_stats", [P, 2, 2])]:
            dbg[nm] = nc.dram_tensor(nm, shp, f32, kind="ExternalOutput").ap()

    s_part = nc.dram_tensor("s_part", [64, 512], f32).ap()
    s_full = nc.dram_tensor("s_full", [64, 512], f32, addr_space="Shared").ap()
    st_part = nc.dram_tensor("st_part", [P, 4], f32).ap()
    st_full = nc.dram_tensor("st_full", [P, 4], f32, addr_space="Shared").ap()
    if dummy_ar:
        dum_in = nc.dram_tensor("dum_in", [1, 128], f32).ap()
        dum_out = nc.dram_tensor("dum_out", [1, 128], f32, addr_space="Shared").ap()

    with SplitWaitTileContext(nc) as tc:
        with tc.tile_pool(name="persist", bufs=1) as pp, \
             tc.tile_pool(name="scratch", bufs=3) as sp, \
             tc.tile_pool(name="psS", bufs=1, space="PSUM") as psS:

            if dummy_ar:
                # fire a no-dependency collective immediately: absorbs the
                # per-execution collective setup / core-alignment cost while
                # the compute engines run the kv/q stages.
                nc.gpsimd.collective_compute(
                    "AllReduce", ALU.add, replica_groups=RG,
                    ins=[dum_in], outs=[dum_out])

            # ---- persistent SBUF loads ----
            cvec_sb = pp.tile([P, 3], f32, tag="cvec_sb")
            nc.sync.dma_start(cvec_sb[:], cvec)
            for ci, cval in enumerate((SMOOTH, BN_EPS, 1.0 / N)):
                nc.const_aps.aps[(f32, cval)] = cvec_sb[:, ci:ci + 1]
            xs_sb = pp.tile([P, 2, SL], mmdt, tag="xs_sb")
            nc.sync.dma_start(xs_sb[:], bc(xs.rearrange("(kt p) n -> p kt n", p=P)))
            wq_sb = pp.tile([P, 2, INNER], mmdt, tag="wq_sb")
            nc.gpsimd.dma_start(wq_sb[:], bc(wq.rearrange("(kt p) e -> p kt e", p=P)))
            wk_sb = pp.tile([P, 2, INNER], mmdt, tag="wk_sb")
            nc.scalar.dma_start(wk_sb[:], bc(wk.rearrange("(kt p) e -> p kt e", p=P)))
            wv_sb = pp.tile([P, 2, INNER], mmdt, tag="wv_sb")
            nc.gpsimd.dma_start(wv_sb[:], bc(wv.rearrange("(kt p) e -> p kt e", p=P)))
            wo_sb = pp.tile([P, 4, C], mmdt, tag="wo_sb")
            nc.gpsimd.dma_start(wo_sb[:], bc(wo.rearrange("(kt p) c -> p kt c", p=P)))
            ind_sb = pp.tile([P, P], mmdt, tag="ind_sb")
            nc.gpsimd.dma_start(ind_sb[:], bc(ind))
            g2_sb = pp.tile([P, 2], f32, tag="g2_sb")
            nc.gpsimd.dma_start(g2_sb[:], gamma2)
            b2_sb = pp.tile([P, 2], f32, tag="b2_sb")
            nc.gpsimd.dma_start(b2_sb[:], beta2)

            khat = pp.tile([P, 4, HEADS, D], mmdt, tag="khat")
            vt_sb = pp.tile([P, 4, INNER], mmdt, tag="vt_sb")
            qhat = pp.tile([P, 4, SL], mmdt, tag="qhat")
            o_sb = pp.tile([P, 4, SL], mmdt, tag="o_sb")
            s_sb = pp.tile([64, HEADS, D], f32, tag="s_sb")
            sfull_sb = pp.tile([P, HEADS, D], mmdt, tag="sfull_sb")
            ys_sb = pp.tile([P, 2, SL], f32, tag="ys_sb")
            yo_sb = pp.tile([P, 2, SL], f32, tag="yo_sb")
            stats_sb = pp.tile([P, 2, 2], f32, tag="stats_sb")
            stf_sb = pp.tile([P, 2, 2], f32, tag="stf_sb")

            # ---- stage KV + S partials (m = this core's 512 positions) ----
            # one PSUM bank per 128-col S region (matmul dst must be
            # bank-aligned; column-offset dst regions corrupt the bank)
            psum_S = [psS.tile([P, P], f32, tag=f"psum_S{pc}", name=f"psum_S{pc}")
                      for pc in range(4)]
            with tc.tile_pool(name="pskv", bufs=2, space="PSUM") as pskv:
                for mc in range(4):
                    xsl = xs_sb[:, :, mc * P:(mc + 1) * P]
                    psum_kT = pskv.tile([P, INNER], f32, tag="psum_kT")
                    psum_vT = pskv.tile([P, INNER], f32, tag="psum_vT")
                    for kt in range(2):
                        nc.tensor.matmul(psum_kT[:], mm(xsl[:, kt]), mm(wk_sb[:, kt]),
                                         start=(kt == 0), stop=(kt == 1))
                    for kt in range(2):
                        nc.tensor.matmul(psum_vT[:], mm(xsl[:, kt]), mm(wv_sb[:, kt]),
                                         start=(kt == 0), stop=(kt == 1))
                    # row norms of kT per head -> normalize
                    k2 = sp.tile([P, HEADS, D], f32, tag="k2")
                    nc.scalar.activation(k2[:], psum_kT.rearrange("p (g d) -> p g d", g=HEADS),
                                         AF.Square)
                    kn2 = sp.tile([P, HEADS], f32, tag="kn2")
                    nc.vector.tensor_reduce(kn2[:], k2[:], axis=mybir.AxisListType.X,
                                            op=ALU.add)
                    kn = sp.tile([P, HEADS], f32, tag="kn")
                    nc.scalar.activation(kn[:], kn2[:], AF.Sqrt, bias=SMOOTH)
                    kninv = sp.tile([P, HEADS], f32, tag="kninv")
                    nc.vector.reciprocal(kninv[:], kn[:])
                    nc.vector.tensor_tensor(
                        khat[:, mc], psum_kT.rearrange("p (g d) -> p g d", g=HEADS),
                        kninv[:, :, None].to_broadcast((P, HEADS, D)), ALU.mult)
                    nc.scalar.activation(vt_sb[:, mc], psum_vT[:], AF.Copy)
                    # S partial accumulation: 4 column regions of one bank
                    kh_flat = khat[:, mc].rearrange("p g d -> p (g d)")
                    for pc in range(4):
                        nc.tensor.matmul(
                            psum_S[pc][:],
                            mm(kh_flat[:, pc * P:(pc + 1) * P]),
                            mm(vt_sb[:, mc, pc * P:(pc + 1) * P]),
                            start=(mc == 0), stop=(mc == 3))

            for pc in range(4):
                nc.vector.tensor_copy(s_sb[:, 2 * pc, :], psum_S[pc][0:64, 0:64])
                nc.vector.tensor_copy(s_sb[:, 2 * pc + 1, :],
                                      psum_S[pc][64:128, 64:128])
            nc.sync.dma_start(s_part, s_sb.rearrange("p g d -> p (g d)"))
            if debug_out:
                nc.sync.dma_start(dbg["d_khat"], khat.bitcast(f32))
                nc.sync.dma_start(dbg["d_vt"], vt_sb.bitcast(f32))
                nc.sync.dma_start(dbg["d_spart"], s_sb[:])
            nc.gpsimd.collective_compute(
                "AllReduce", ALU.add, replica_groups=RG,
                ins=[s_part], outs=[s_full])
            sf_r = s_full.rearrange("p (g d) -> p g d", g=HEADS)
            nc.sync.dma_start(sfull_sb[0:64], bc(sf_r))
            nc.sync.dma_start(sfull_sb[64:128], bc(sf_r))
            if debug_out:
                nc.sync.dma_start(dbg["d_sfull"], sfull_sb.bitcast(f32))

            # ---- stage Q (n = this core's 512 positions) ----
            with tc.tile_pool(name="psq", bufs=2, space="PSUM") as psq:
                for pc in range(4):
                    psum_q = psq.tile([P, SL], f32, tag="psum_q")
                    for kt in range(2):
                        nc.tensor.matmul(psum_q[:],
                                         mm(wq_sb[:, kt, pc * P:(pc + 1) * P]),
                                         mm(xs_sb[:, kt]),
                                         start=(kt == 0), stop=(kt == 1))
                    q2 = sp.tile([P, SL], mmdt, tag="q2")
                    nc.scalar.activation(q2[:], psum_q[:], AF.Square)
                    psum_nrm = psq.tile([P, SL], f32, tag="psum_nrm")
                    nc.tensor.matmul(psum_nrm[:], mm(ind_sb[:]), mm(q2[:]),
                                     start=True, stop=True)
                    qn = sp.tile([P, SL], f32, tag="qn")
                    nc.scalar.activation(qn[:], psum_nrm[:], AF.Sqrt, bias=SMOOTH)
                    qninv = sp.tile([P, SL], f32, tag="qninv")
                    nc.vector.reciprocal(qninv[:], qn[:])
                    nc.vector.tensor_tensor(qhat[:, pc], psum_q[:], qninv[:], ALU.mult)

            # ---- stage O: o[e,n] = sum_e' S[e',e] qhat[e',n], 2 heads/chunk ----
            with tc.tile_pool(name="pso", bufs=2, space="PSUM") as pso:
                for pc in range(4):
                    # two heads per chunk; PE row-groups 0-1 and 2-3 run
                    # these K=64 matmuls concurrently (row packing)
                    psum_oa = pso.tile([64, SL], f32, tag="psum_oa")
                    psum_ob = pso.tile([64, SL], f32, tag="psum_ob")
                    nc.tensor.matmul(psum_oa[:],
                                     mm(sfull_sb[0:64, 2 * pc, :]),
                                     mm(qhat[0:64, pc]),
                                     start=True, stop=True)
                    nc.tensor.matmul(psum_ob[:],
                                     mm(sfull_sb[64:128, 2 * pc + 1, :]),
                                     mm(qhat[64:128, pc]),
                                     start=True, stop=True)
                    nc.scalar.activation(o_sb[0:64, pc], psum_oa[:], AF.Copy)
                    nc.vector.tensor_copy(o_sb[64:128, pc], psum_ob[:])

            # ---- stage Y: y = w_out @ o, BN stats partials ----
            with tc.tile_pool(name="psy", bufs=2, space="PSUM") as psy:
                for cc in range(2):
                    psum_y = psy.tile([P, SL], f32, tag="psum_y")
                    for kt in range(4):
                        nc.tensor.matmul(psum_y[:],
                                         mm(wo_sb[:, kt, cc * P:(cc + 1) * P]),
                                         mm(o_sb[:, kt]),
                                         start=(kt == 0), stop=(kt == 3))
                    nc.scalar.activation(ys_sb[:, cc], psum_y[:], AF.Copy,
                                         accum_out=stats_sb[:, cc, 0:1])
                    junk = sp.tile([P, SL], f32, tag="junk")
                    nc.scalar.activation(junk[:], psum_y[:], AF.Square,
                                         accum_out=stats_sb[:, cc, 1:2])

            if debug_out:
                nc.sync.dma_start(dbg["d_qhat"], qhat.bitcast(f32))
                nc.sync.dma_start(dbg["d_o"], o_sb.bitcast(f32))
                nc.sync.dma_start(dbg["d_ys"], ys_sb[:])
                nc.sync.dma_start(dbg["d_stats"], stats_sb[:])
            nc.sync.dma_start(st_part, stats_sb.rearrange("p a b -> p (a b)"))
            nc.gpsimd.collective_compute(
                "AllReduce", ALU.add, replica_groups=RG,
                ins=[st_part], outs=[st_full])
            nc.sync.dma_start(stf_sb.rearrange("p a b -> p (a b)"), st_full)

            # ---- BN finalize: scale/shift per channel ----
            me = pp.tile([P, 2, 2], f32, tag="me")
            nc.scalar.activation(me[:], stf_sb[:], AF.Copy, scale=1.0 / N)
            mean = me[:, :, 0]
            ex2 = me[:, :, 1]
            var = pp.tile([P, 2], f32, tag="var")
            nc.vector.tensor_tensor(var[:], mean, mean, ALU.mult)
            nc.vector.tensor_tensor(var[:], ex2, var[:], ALU.subtract)
            std = pp.tile([P, 2], f32, tag="std")
            nc.scalar.activation(std[:], var[:], AF.Sqrt, bias=BN_EPS)
            rstd = pp.tile([P, 2], f32, tag="rstd")
            nc.vector.reciprocal(rstd[:], std[:])
            scale = pp.tile([P, 2], f32, tag="scale")
            nc.vector.tensor_tensor(scale[:], g2_sb[:], rstd[:], ALU.mult)
            shift = pp.tile([P, 2], f32, tag="shift")
            nc.vector.tensor_tensor(shift[:], mean[:], scale[:], ALU.mult)
            nc.vector.tensor_tensor(shift[:], b2_sb[:], shift[:], ALU.subtract)

            # ---- apply BN + ReLU, store ----
            y_r = y.rearrange("(cc p) n -> p cc n", p=P)
            for cc in range(2):
                nc.scalar.activation(yo_sb[:, cc], ys_sb[:, cc], AF.Relu,
                                     bias=shift[:, cc:cc + 1],
                                     scale=scale[:, cc:cc + 1])
                nc.sync.dma_start(y_r[:, cc], yo_sb[:, cc])

    return nc


def _prep_inputs(x, w_qkv, w_out, gamma, beta):
    X = np.ascontiguousarray(x.reshape(C, N))
    wq = np.ascontiguousarray(w_qkv[0:INNER].T)
    wk = np.ascontiguousarray(w_qkv[INNER:2 * INNER].T)
    wv = np.ascontiguousarray(w_qkv[2 * INNER:3 * INNER].T)
    wo = np.ascontiguousarray(w_out.T)
    r = np.arange(P)
    ind = (((r[:, None] < D) == (r[None, :] < D))).astype(np.float32)
    ind = np.ascontiguousarray(ind)
    gamma2 = np.ascontiguousarray(gamma.reshape(2, P).T)
    beta2 = np.ascontiguousarray(beta.reshape(2, P).T)
    cvec = np.tile(np.array([SMOOTH, BN_EPS, 1.0 / N], np.float32), (P, 1))
    common = dict(wq=wq, wk=wk, wv=wv, wo=wo, ind=ind, gamma2=gamma2,
                  beta2=beta2, cvec=np.ascontiguousarray(cvec))
    in_maps = []
    for i in range(NCORES):
        m = dict(common)
        m["xs"] = np.ascontiguousarray(X[:, i * SL:(i + 1) * SL])
        in_maps.append(m)
    return in_maps


def kernel(x, w_qkv, w_out, gamma, beta):
    from concourse.bass_utils import run_bass_kernel_spmd

    if "nc" not in _CACHE:
        _CACHE["nc"] = build_nc()
    nc = _CACHE["nc"]

    in_maps = _prep_inputs(
        np.asarray(x, dtype=np.float32),
        np.asarray(w_qkv, dtype=np.float32),
        np.asarray(w_out, dtype=np.float32),
        np.asarray(gamma, dtype=np.float32),
        np.asarray(beta, dtype=np.float32),
    )
    res = run_bass_kernel_spmd(nc, in_maps, list(range(NCORES)))
    out = np.concatenate([res.results[i]["y"] for i in range(NCORES)], axis=1)
    return out.reshape(1, C, 64, 64).astype(np.float32)


def run_traced(x, w_qkv, w_out, gamma, beta, **kw):
    """Like kernel() but with NTFF tracing; returns (output, BassKernelResults)."""
    from concourse.bass_utils import run_bass_kernel_spmd

    if "nc" not in _CACHE:
        _CACHE["nc"] = build_nc()
    nc = _CACHE["nc"]
    in_maps = _prep_inputs(np.asarray(x, np.float32), np.asarray(w_qkv, np.float32),
                           np.asarray(w_out, np.float32), np.asarray(gamma, np.float32),
                           np.asarray(beta, np.float32))
    res = run_bass_kernel_spmd(nc, in_maps, list(range(NCORES)), trace=True, **kw)
    out = np.concatenate([res.results[i]["y"] for i in range(NCORES)], axis=1)
    return out.reshape(1, C, 64, 64).astype(np.float32), res


# revision 19
# speedup vs baseline: 1.1040x; 1.0290x over previous
"""Trainium2 Bass kernel for nn_CNNAttention (cosine-normalized linear attention).

Math: the reference "attention" has no softmax -- after folding the cosine
normalization into q^ = q/sqrt(|q|^2+eps), k^ = k/sqrt(|k|^2+eps) (the +eps
on the attn denominator is a ~2e-6 relative perturbation, far below fp32
matmul noise), the computation is linear and reassociates:

    o = (q^T k^  / norm) v   ==   o[d,n] = sum_e S[e,d] q^[e,n],
    S = sum_m k^T[m,e] v^T[m,d]            (64x64 per head)

so no 4096x4096 attention matrix is ever materialized.

Sharding: each of the 8 cores owns 512 of the 4096 spatial positions, both
as its q/o/y slice and as its k/v slice for the S partial contraction.
Cross-core comms: one 256KB AllReduce for S, one 2KB AllReduce for BN stats.
"""

import numpy as np

HEADS = 8
D = 64
C = 256
N = 4096
NCORES = 8
SL = N // NCORES      # 512 spatial positions per core
INNER = HEADS * D     # 512
SMOOTH = 1e-4
BN_EPS = 1e-5
P = 128

# tunables
MM_DTYPE = "float32r"   # "float32" (4 cyc/row, exact) or "float32r" (1 cyc/row)
DUMMY_WARMUP_AR = False  # absorb first-collective setup cost concurrently w/ compute

_CACHE = {}


def _make_tile_context_cls(tile, mybir, ScopedClock):
    class SplitWaitTileContext(tile.TileContext):
        """Workaround: this walrus build accepts only ONE sync-wait command
        per instruction. Tile attaches one wait per cross-proc dependency.
        After scheduling, splice all-but-one wait of each multi-wait
        instruction onto single-wait NOPs inserted just before it on the
        same engine (engine streams execute in order, so semantics are
        preserved)."""

        def __exit__(self, exc_type, exc_val, exc_tb):
            r = super().__exit__(exc_type, exc_val, exc_tb)
            if exc_type is None:
                self._split_multi_waits()
            return r

        def _drain_and_barrier(self, tick_clock, wait_clock):
            nc = self.nc
            drain_inst = nc.sync.drain()
            wait_clock.add_sem_waits(
                drain_inst.ins, ScopedClock({None: tick_clock.global_clock})
            )
            nc.all_engine_barrier()
            assert self.sems is not None
            popped = nc._tile_sem_poison_stack.pop()
            assert popped is self._sem_poison
            nc.clear_and_free_semaphores(list(self.sems.allocated().values()))

        def _split_multi_waits(self):
            nc = self.nc
            k = 0
            for bass_bb in nc.main_func.blocks:
                insts = bass_bb.instructions
                i = 0
                while i < len(insts):
                    inst = insts[i]
                    si = inst.sync_info
                    waits = list(si.on_wait) if si and si.on_wait else []
                    if len(waits) > 1:
                        si.on_wait = [waits[-1]]
                        for w in waits[:-1]:
                            nop = mybir.InstNoOp(
                                name=f"I-waitsplit-{k}", ins=[], outs=[],
                                text_hint="waitsplit", bass_nofuse=True,
                            )
                            nop.engine = inst.engine
                            nop.sync_info = mybir.SyncInfo(
                                on_wait=[w], on_update=[])
                            insts.insert(i, nop)
                            i += 1
                            k += 1
                    i += 1

    return SplitWaitTileContext


def build_nc(mm_dtype=MM_DTYPE, dummy_ar=DUMMY_WARMUP_AR, debug_out=False):
    import concourse.bass as bass
    import concourse.mybir as mybir
    import concourse.tile as tile
    from concourse.vector_clock import ScopedClock

    SplitWaitTileContext = _make_tile_context_cls(tile, mybir, ScopedClock)

    f32 = mybir.dt.float32
    mmdt = getattr(mybir.dt, mm_dtype)
    AF = mybir.ActivationFunctionType
    ALU = mybir.AluOpType
    RG = [list(range(NCORES))]

    def mm(ap):
        return ap

    def bc(ap):
        # bitcast a DRAM f32 AP to the matmul dtype (raw 4-byte copy)
        return ap if mmdt == f32 else ap.bitcast(mmdt)

    nc = bass.Bass("TRN2", target_bir_lowering=False, debug=False,
                   num_devices=NCORES)

    cvec = nc.dram_tensor("cvec", [P, 3], f32, kind="ExternalInput").ap()
    xs = nc.dram_tensor("xs", [C, SL], f32, kind="ExternalInput").ap()
    wq = nc.dram_tensor("wq", [C, INNER], f32, kind="ExternalInput").ap()
    wk = nc.dram_tensor("wk", [C, INNER], f32, kind="ExternalInput").ap()
    wv = nc.dram_tensor("wv", [C, INNER], f32, kind="ExternalInput").ap()
    wo = nc.dram_tensor("wo", [INNER, C], f32, kind="ExternalInput").ap()
    ind = nc.dram_tensor("ind", [P, P], f32, kind="ExternalInput").ap()
    gamma2 = nc.dram_tensor("gamma2", [P, 2], f32, kind="ExternalInput").ap()
    beta2 = nc.dram_tensor("beta2", [P, 2], f32, kind="ExternalInput").ap()
    y = nc.dram_tensor("y", [C, SL], f32, kind="ExternalOutput").ap()
    dbg = {}
    if debug_out:
        for nm, shp in [("d_khat", [P, 4, HEADS, D]), ("d_vt", [P, 4, INNER]),
                        ("d_qhat", [P, 4, SL]), ("d_spart", [64, HEADS, D]),
                        ("d_sfull", [P, HEADS, D]), ("d_o", [P, 4, SL]),
                        ("d_ys", [P, 2, SL]), ("d_stats", [P, 2, 2])]:
            dbg[nm] = nc.dram_tensor(nm, shp, f32, kind="ExternalOutput").ap()

    s_part = nc.dram_tensor("s_part", [64, 512], f32).ap()
    s_full = nc.dram_tensor("s_full", [64, 512], f32, addr_space="Shared").ap()
    st_part = nc.dram_tensor("st_part", [P, 4], f32).ap()
    st_full = nc.dram_tensor("st_full", [P, 4], f32, addr_space="Shared").ap()
    if dummy_ar:
        dum_in = nc.dram_tensor("dum_in", [1, 128], f32).ap()
        dum_out = nc.dram_tensor("dum_out", [1, 128], f32, addr_space="Shared").ap()

    with SplitWaitTileContext(nc) as tc:
        with tc.tile_pool(name="persist", bufs=1) as pp, \
             tc.tile_pool(name="scratch", bufs=3) as sp, \
             tc.tile_pool(name="psS", bufs=1, space="PSUM") as psS:

            if dummy_ar:
                # fire a no-dependency collective immediately: absorbs the
                # per-execution collective setup / core-alignment cost while
                # the compute engines run the kv/q stages.
                nc.gpsimd.collective_compute(
                    "AllReduce", ALU.add, replica_groups=RG,
                    ins=[dum_in], outs=[dum_out])

            # ---- persistent SBUF loads ----
            cvec_sb = pp.tile([P, 3], f32, tag="cvec_sb")
            nc.sync.dma_start(cvec_sb[:], cvec)
            for ci, cval in enumerate((SMOOTH, BN_EPS, 1.0 / N)):
                nc.const_aps.aps[(f32, cval)] = cvec_sb[:, ci:ci + 1]
            xs_sb = pp.tile([P, 2, SL], mmdt, tag="xs_sb")
            nc.sync.dma_start(xs_sb[:], bc(xs.rearrange("(kt p) n -> p kt n", p=P)))
            wq_sb = pp.tile([P, 2, INNER], mmdt, tag="wq_sb")
            nc.gpsimd.dma_start(wq_sb[:], bc(wq.rearrange("(kt p) e -> p kt e", p=P)))
            wk_sb = pp.tile([P, 2, INNER], mmdt, tag="wk_sb")
            nc.scalar.dma_start(wk_sb[:], bc(wk.rearrange("(kt p) e -> p kt e", p=P)))
            wv_sb = pp.tile([P, 2, INNER], mmdt, tag="wv_sb")
            nc.gpsimd.dma_start(wv_sb[:], bc(wv.rearrange("(kt p) e -> p kt e", p=P)))
            wo_sb = pp.tile([P, 4, C], mmdt, tag="wo_sb")
            nc.gpsimd.dma_start(wo_sb[:], bc(wo.rearrange("(kt p) c -> p kt c", p=P)))
            ind_sb = pp.tile([P, P], mmdt, tag="ind_sb")
            nc.gpsimd.dma_start(ind_sb[:], bc(ind))
            g2_sb = pp.tile([P, 2], f32, tag="g2_sb")
            nc.gpsimd.dma_start(g2_sb[:], gamma2)
            b2_sb = pp.tile([P, 2], f32, tag="b2_sb")
            nc.gpsimd.dma_start(b2_sb[:], beta2)

            khat = pp.tile([P, 4, HEADS, D], mmdt, tag="khat")
            vt_sb = pp.tile([P, 4, INNER], mmdt, tag="vt_sb")
            qhat = pp.tile([P, 4, SL], mmdt, tag="qhat")
            o_sb = pp.tile([P, 4, SL], mmdt, tag="o_sb")
            s_sb = pp.tile([64, HEADS, D], f32, tag="s_sb")
            sfull_sb = pp.tile([P, HEADS, D], mmdt, tag="sfull_sb")
            ys_sb = pp.tile([P, 2, SL], f32, tag="ys_sb")
            yo_sb = pp.tile([P, 2, SL], f32, tag="yo_sb")
            stats_sb = pp.tile([P, 2, 2], f32, tag="stats_sb")
            stf_sb = pp.tile([P, 2, 2], f32, tag="stf_sb")

            # ---- stage KV + S partials (m = this core's 512 positions) ----
            # one PSUM bank per 128-col S region (matmul dst must be
            # bank-aligned; column-offset dst regions corrupt the bank)
            psum_S = [psS.tile([P, P], f32, tag=f"psum_S{pc}", name=f"psum_S{pc}")
                      for pc in range(4)]
            with tc.tile_pool(name="pskv", bufs=2, space="PSUM") as pskv:
                for mc in range(4):
                    xsl = xs_sb[:, :, mc * P:(mc + 1) * P]
                    psum_kT = pskv.tile([P, INNER], f32, tag="psum_kT")
                    psum_vT = pskv.tile([P, INNER], f32, tag="psum_vT")
                    for kt in range(2):
                        nc.tensor.matmul(psum_kT[:], mm(xsl[:, kt]), mm(wk_sb[:, kt]),
                                         start=(kt == 0), stop=(kt == 1))
                    for kt in range(2):
                        nc.tensor.matmul(psum_vT[:], mm(xsl[:, kt]), mm(wv_sb[:, kt]),
                                         start=(kt == 0), stop=(kt == 1))
                    # row norms of kT per head -> normalize
                    k2 = sp.tile([P, HEADS, D], f32, tag="k2")
                    nc.scalar.activation(k2[:], psum_kT.rearrange("p (g d) -> p g d", g=HEADS),
                                         AF.Square)
                    kn2 = sp.tile([P, HEADS], f32, tag="kn2")
                    nc.vector.tensor_reduce(kn2[:], k2[:], axis=mybir.AxisListType.X,
                                            op=ALU.add)
                    kn = sp.tile([P, HEADS], f32, tag="kn")
                    nc.scalar.activation(kn[:], kn2[:], AF.Sqrt, bias=SMOOTH)
                    kninv = sp.tile([P, HEADS], f32, tag="kninv")
                    nc.vector.reciprocal(kninv[:], kn[:])
                    nc.vector.tensor_tensor(
                        khat[:, mc], psum_kT.rearrange("p (g d) -> p g d", g=HEADS),
                        kninv[:, :, None].to_broadcast((P, HEADS, D)), ALU.mult)
                    nc.scalar.activation(vt_sb[:, mc], psum_vT[:], AF.Copy)
                    # S partial accumulation: 4 column regions of one bank
                    kh_flat = khat[:, mc].rearrange("p g d -> p (g d)")
                    for pc in range(4):
                        nc.tensor.matmul(
                            psum_S[pc][:],
                            mm(kh_flat[:, pc * P:(pc + 1) * P]),
                            mm(vt_sb[:, mc, pc * P:(pc + 1) * P]),
                            start=(mc == 0), stop=(mc == 3))

            for pc in range(4):
                nc.vector.tensor_copy(s_sb[:, 2 * pc, :], psum_S[pc][0:64, 0:64])
                nc.vector.tensor_copy(s_sb[:, 2 * pc + 1, :],
                                      psum_S[pc][64:128, 64:128])
            nc.sync.dma_start(s_part, s_sb.rearrange("p g d -> p (g d)"))
            if debug_out:
                nc.sync.dma_start(dbg["d_khat"], khat.bitcast(f32))
                nc.sync.dma_start(dbg["d_vt"], vt_sb.bitcast(f32))
                nc.sync.dma_start(dbg["d_spart"], s_sb[:])
            nc.gpsimd.collective_compute(
                "AllReduce", ALU.add, replica_groups=RG,
                ins=[s_part], outs=[s_full])
            sf_r = s_full.rearrange("p (g d) -> p g d", g=HEADS)
            nc.sync.dma_start(sfull_sb[0:64], bc(sf_r))
            nc.scalar.dma_start(sfull_sb[64:128], bc(sf_r))
            if debug_out:
                nc.sync.dma_start(dbg["d_sfull"], sfull_sb.bitcast(f32))

            # ---- stage Q (n = this core's 512 positions) ----
            with tc.tile_pool(name="psq", bufs=2, space="PSUM") as psq:
                for pc in range(4):
                    psum_q = psq.tile([P, SL], f32, tag="psum_q")
                    for kt in range(2):
                        nc.tensor.matmul(psum_q[:],
                                         mm(wq_sb[:, kt, pc * P:(pc + 1) * P]),
                                         mm(xs_sb[:, kt]),
                                         start=(kt == 0), stop=(kt == 1))
                    q2 = sp.tile([P, SL], mmdt, tag="q2")
                    nc.scalar.activation(q2[:], psum_q[:], AF.Square)
                    psum_nrm = psq.tile([P, SL], f32, tag="psum_nrm")
                    nc.tensor.matmul(psum_nrm[:], mm(ind_sb[:]), mm(q2[:]),
                                     start=True, stop=True)
                    qn = sp.tile([P, SL], f32, tag="qn")
                    nc.scalar.activation(qn[:], psum_nrm[:], AF.Sqrt, bias=SMOOTH)
                    qninv = sp.tile([P, SL], f32, tag="qninv")
                    nc.vector.reciprocal(qninv[:], qn[:])
                    nc.vector.tensor_tensor(qhat[:, pc], psum_q[:], qninv[:], ALU.mult)

            # ---- stage O: o[e,n] = sum_e' S[e',e] qhat[e',n], 2 heads/chunk ----
            with tc.tile_pool(name="pso", bufs=2, space="PSUM") as pso:
                for pc in range(4):
                    # two heads per chunk; PE row-groups 0-1 and 2-3 run
                    # these K=64 matmuls concurrently (row packing)
                    psum_oa = pso.tile([64, SL], f32, tag="psum_oa")
                    psum_ob = pso.tile([64, SL], f32, tag="psum_ob")
                    nc.tensor.matmul(psum_oa[:],
                                     mm(sfull_sb[0:64, 2 * pc, :]),
                                     mm(qhat[0:64, pc]),
                                     start=True, stop=True)
                    nc.tensor.matmul(psum_ob[:],
                                     mm(sfull_sb[64:128, 2 * pc + 1, :]),
                                     mm(qhat[64:128, pc]),
                                     start=True, stop=True)
                    nc.scalar.activation(o_sb[0:64, pc], psum_oa[:], AF.Copy)
                    nc.vector.tensor_copy(o_sb[64:128, pc], psum_ob[:])

            # ---- stage Y: y = w_out @ o, BN stats partials ----
            with tc.tile_pool(name="psy", bufs=2, space="PSUM") as psy:
                for cc in range(2):
                    psum_y = psy.tile([P, SL], f32, tag="psum_y")
                    for kt in range(4):
                        nc.tensor.matmul(psum_y[:],
                                         mm(wo_sb[:, kt, cc * P:(cc + 1) * P]),
                                         mm(o_sb[:, kt]),
                                         start=(kt == 0), stop=(kt == 3))
                    nc.scalar.activation(ys_sb[:, cc], psum_y[:], AF.Copy,
                                         accum_out=stats_sb[:, cc, 0:1])
                    junk = sp.tile([P, SL], f32, tag="junk")
                    nc.scalar.activation(junk[:], psum_y[:], AF.Square,
                                         accum_out=stats_sb[:, cc, 1:2])

            if debug_out:
                nc.sync.dma_start(dbg["d_qhat"], qhat.bitcast(f32))
                nc.sync.dma_start(dbg["d_o"], o_sb.bitcast(f32))
                nc.sync.dma_start(dbg["d_ys"], ys_sb[:])
                nc.sync.dma_start(dbg["d_stats"], stats_sb[:])
            nc.sync.dma_start(st_part, stats_sb.rearrange("p a b -> p (a b)"))
            nc.gpsimd.collective_compute(
                "AllReduce", ALU.add, replica_groups=RG,
                ins=[st_part], outs=[st_full])
            nc.sync.dma_start(stf_sb.rearrange("p a b -> p (a b)"), st_full)

            # ---- BN finalize: scale/shift per channel ----
            me = pp.tile([P, 2, 2], f32, tag="me")
            nc.scalar.activation(me[:], stf_sb[:], AF.Copy, scale=1.0 / N)
            mean = me[:, :, 0]
            ex2 = me[:, :, 1]
            var = pp.tile([P, 2], f32, tag="var")
            nc.vector.tensor_tensor(var[:], mean, mean, ALU.mult)
            nc.vector.tensor_tensor(var[:], ex2, var[:], ALU.subtract)
            std = pp.tile([P, 2], f32, tag="std")
            nc.scalar.activation(std[:], var[:], AF.Sqrt, bias=BN_EPS)
            rstd = pp.tile([P, 2], f32, tag="rstd")
            nc.vector.reciprocal(rstd[:], std[:])
            scale = pp.tile([P, 2], f32, tag="scale")
            nc.vector.tensor_tensor(scale[:], g2_sb[:], rstd[:], ALU.mult)
            shift = pp.tile([P, 2], f32, tag="shift")
            nc.vector.tensor_tensor(shift[:], mean[:], scale[:], ALU.mult)
            nc.vector.tensor_tensor(shift[:], b2_sb[:], shift[:], ALU.subtract)

            # ---- apply BN + ReLU, store ----
            y_r = y.rearrange("(cc p) n -> p cc n", p=P)
            for cc in range(2):
                nc.scalar.activation(yo_sb[:, cc], ys_sb[:, cc], AF.Relu,
                                     bias=shift[:, cc:cc + 1],
                                     scale=scale[:, cc:cc + 1])
                nc.sync.dma_start(y_r[:, cc], yo_sb[:, cc])

    return nc


def _prep_inputs(x, w_qkv, w_out, gamma, beta):
    X = np.ascontiguousarray(x.reshape(C, N))
    wq = np.ascontiguousarray(w_qkv[0:INNER].T)
    wk = np.ascontiguousarray(w_qkv[INNER:2 * INNER].T)
    wv = np.ascontiguousarray(w_qkv[2 * INNER:3 * INNER].T)
    wo = np.ascontiguousarray(w_out.T)
    r = np.arange(P)
    ind = (((r[:, None] < D) == (r[None, :] < D))).astype(np.float32)
    ind = np.ascontiguousarray(ind)
    gamma2 = np.ascontiguousarray(gamma.reshape(2, P).T)
    beta2 = np.ascontiguousarray(beta.reshape(2, P).T)
    cvec = np.tile(np.array([SMOOTH, BN_EPS, 1.0 / N], np.float32), (P, 1))
    common = dict(wq=wq, wk=wk, wv=wv, wo=wo, ind=ind, gamma2=gamma2,
                  beta2=beta2, cvec=np.ascontiguousarray(cvec))
    in_maps = []
    for i in range(NCORES):
        m = dict(common)
        m["xs"] = np.ascontiguousarray(X[:, i * SL:(i + 1) * SL])
        in_maps.append(m)
    return in_maps


def kernel(x, w_qkv, w_out, gamma, beta):
    from concourse.bass_utils import run_bass_kernel_spmd

    if "nc" not in _CACHE:
        _CACHE["nc"] = build_nc()
    nc = _CACHE["nc"]

    in_maps = _prep_inputs(
        np.asarray(x, dtype=np.float32),
        np.asarray(w_qkv, dtype=np.float32),
        np.asarray(w_out, dtype=np.float32),
        np.asarray(gamma, dtype=np.float32),
        np.asarray(beta, dtype=np.float32),
    )
    res = run_bass_kernel_spmd(nc, in_maps, list(range(NCORES)))
    out = np.concatenate([res.results[i]["y"] for i in range(NCORES)], axis=1)
    return out.reshape(1, C, 64, 64).astype(np.float32)


def run_traced(x, w_qkv, w_out, gamma, beta, **kw):
    """Like kernel() but with NTFF tracing; returns (output, BassKernelResults)."""
    from concourse.bass_utils import run_bass_kernel_spmd

    if "nc" not in _CACHE:
        _CACHE["nc"] = build_nc()
    nc = _CACHE["nc"]
    in_maps = _prep_inputs(np.asarray(x, np.float32), np.asarray(w_qkv, np.float32),
                           np.asarray(w_out, np.float32), np.asarray(gamma, np.float32),
                           np.asarray(beta, np.float32))
    res = run_bass_kernel_spmd(nc, in_maps, list(range(NCORES)), trace=True, **kw)
    out = np.concatenate([res.results[i]["y"] for i in range(NCORES)], axis=1)
    return out.reshape(1, C, 64, 64).astype(np.float32), res


# revision 20
# speedup vs baseline: 1.1613x; 1.0519x over previous
"""Trainium2 Bass kernel for nn_CNNAttention (cosine-normalized linear attention).

Math: the reference "attention" has no softmax -- after folding the cosine
normalization into q^ = q/sqrt(|q|^2+eps), k^ = k/sqrt(|k|^2+eps) (the +eps
on the attn denominator is a ~2e-6 relative perturbation, far below fp32
matmul noise), the computation is linear and reassociates:

    o = (q^T k^  / norm) v   ==   o[d,n] = sum_e S[e,d] q^[e,n],
    S = sum_m k^T[m,e] v^T[m,d]            (64x64 per head)

so no 4096x4096 attention matrix is ever materialized.

Sharding: each of the 8 cores owns 512 of the 4096 spatial positions, both
as its q/o/y slice and as its k/v slice for the S partial contraction.
Cross-core comms: one 256KB AllReduce for S, one 2KB AllReduce for BN stats.
"""

import numpy as np

HEADS = 8
D = 64
C = 256
N = 4096
NCORES = 8
SL = N // NCORES      # 512 spatial positions per core
INNER = HEADS * D     # 512
SMOOTH = 1e-4
BN_EPS = 1e-5
P = 128

# tunables
MM_DTYPE = "float32r"   # "float32" (4 cyc/row, exact) or "float32r" (1 cyc/row)
DUMMY_WARMUP_AR = False  # absorb first-collective setup cost concurrently w/ compute

_CACHE = {}


def _make_tile_context_cls(tile, mybir, ScopedClock):
    class SplitWaitTileContext(tile.TileContext):
        """Workaround: this walrus build accepts only ONE sync-wait command
        per instruction. Tile attaches one wait per cross-proc dependency.
        After scheduling, splice all-but-one wait of each multi-wait
        instruction onto single-wait NOPs inserted just before it on the
        same engine (engine streams execute in order, so semantics are
        preserved)."""

        def __exit__(self, exc_type, exc_val, exc_tb):
            r = super().__exit__(exc_type, exc_val, exc_tb)
            if exc_type is None:
                self._split_multi_waits()
            return r

        def _drain_and_barrier(self, tick_clock, wait_clock):
            nc = self.nc
            drain_inst = nc.sync.drain()
            wait_clock.add_sem_waits(
                drain_inst.ins, ScopedClock({None: tick_clock.global_clock})
            )
            nc.all_engine_barrier()
            assert self.sems is not None
            popped = nc._tile_sem_poison_stack.pop()
            assert popped is self._sem_poison
            nc.clear_and_free_semaphores(list(self.sems.allocated().values()))

        def _split_multi_waits(self):
            nc = self.nc
            k = 0
            for bass_bb in nc.main_func.blocks:
                insts = bass_bb.instructions
                i = 0
                while i < len(insts):
                    inst = insts[i]
                    si = inst.sync_info
                    waits = list(si.on_wait) if si and si.on_wait else []
                    if len(waits) > 1:
                        si.on_wait = [waits[-1]]
                        for w in waits[:-1]:
                            nop = mybir.InstNoOp(
                                name=f"I-waitsplit-{k}", ins=[], outs=[],
                                text_hint="waitsplit", bass_nofuse=True,
                            )
                            nop.engine = inst.engine
                            nop.sync_info = mybir.SyncInfo(
                                on_wait=[w], on_update=[])
                            insts.insert(i, nop)
                            i += 1
                            k += 1
                    i += 1

    return SplitWaitTileContext


def build_nc(mm_dtype=MM_DTYPE, dummy_ar=DUMMY_WARMUP_AR, debug_out=False):
    import concourse.bass as bass
    import concourse.mybir as mybir
    import concourse.tile as tile
    from concourse.vector_clock import ScopedClock

    SplitWaitTileContext = _make_tile_context_cls(tile, mybir, ScopedClock)

    f32 = mybir.dt.float32
    mmdt = getattr(mybir.dt, mm_dtype)
    AF = mybir.ActivationFunctionType
    ALU = mybir.AluOpType
    RG = [list(range(NCORES))]

    def mm(ap):
        return ap

    def bc(ap):
        # bitcast a DRAM f32 AP to the matmul dtype (raw 4-byte copy)
        return ap if mmdt == f32 else ap.bitcast(mmdt)

    nc = bass.Bass("TRN2", target_bir_lowering=False, debug=False,
                   num_devices=NCORES)

    cvec = nc.dram_tensor("cvec", [P, 3], f32, kind="ExternalInput").ap()
    xs = nc.dram_tensor("xs", [C, SL], f32, kind="ExternalInput").ap()
    wq = nc.dram_tensor("wq", [C, INNER], f32, kind="ExternalInput").ap()
    wk = nc.dram_tensor("wk", [C, INNER], f32, kind="ExternalInput").ap()
    wv = nc.dram_tensor("wv", [C, INNER], f32, kind="ExternalInput").ap()
    wo = nc.dram_tensor("wo", [INNER, C], f32, kind="ExternalInput").ap()
    ind = nc.dram_tensor("ind", [P, P], f32, kind="ExternalInput").ap()
    gamma2 = nc.dram_tensor("gamma2", [P, 2], f32, kind="ExternalInput").ap()
    beta2 = nc.dram_tensor("beta2", [P, 2], f32, kind="ExternalInput").ap()
    y = nc.dram_tensor("y", [C, SL], f32, kind="ExternalOutput").ap()
    dbg = {}
    if debug_out:
        for nm, shp in [("d_khat", [P, 4, HEADS, D]), ("d_vt", [P, 4, INNER]),
                        ("d_qhat", [P, 4, SL]), ("d_spart", [64, HEADS, D]),
                        ("d_sfull", [P, HEADS, D]), ("d_o", [P, 4, SL]),
                        ("d_ys", [P, 2, SL]), ("d_stats", [P, 2, 2])]:
            dbg[nm] = nc.dram_tensor(nm, shp, f32, kind="ExternalOutput").ap()

    s_part = nc.dram_tensor("s_part", [64, 512], f32).ap()
    s_full = nc.dram_tensor("s_full", [64, 512], f32, addr_space="Shared").ap()
    st_part = nc.dram_tensor("st_part", [P, 4], f32).ap()
    st_full = nc.dram_tensor("st_full", [P, 4], f32, addr_space="Shared").ap()
    if dummy_ar:
        dum_in = nc.dram_tensor("dum_in", [1, 128], f32).ap()
        dum_out = nc.dram_tensor("dum_out", [1, 128], f32, addr_space="Shared").ap()

    with SplitWaitTileContext(nc) as tc:
        with tc.tile_pool(name="persist", bufs=1) as pp, \
             tc.tile_pool(name="scratch", bufs=3) as sp, \
             tc.tile_pool(name="psS", bufs=1, space="PSUM") as psS:

            if dummy_ar:
                # fire a no-dependency collective immediately: absorbs the
                # per-execution collective setup / core-alignment cost while
                # the compute engines run the kv/q stages.
                nc.gpsimd.collective_compute(
                    "AllReduce", ALU.add, replica_groups=RG,
                    ins=[dum_in], outs=[dum_out])

            # ---- persistent SBUF loads ----
            cvec_sb = pp.tile([P, 3], f32, tag="cvec_sb")
            nc.sync.dma_start(cvec_sb[:], cvec)
            for ci, cval in enumerate((SMOOTH, BN_EPS, 1.0 / N)):
                nc.const_aps.aps[(f32, cval)] = cvec_sb[:, ci:ci + 1]
            xs_sb = pp.tile([P, 2, SL], mmdt, tag="xs_sb")
            nc.sync.dma_start(xs_sb[:], bc(xs.rearrange("(kt p) n -> p kt n", p=P)))
            wq_sb = pp.tile([P, 2, INNER], mmdt, tag="wq_sb")
            nc.gpsimd.dma_start(wq_sb[:], bc(wq.rearrange("(kt p) e -> p kt e", p=P)))
            wk_sb = pp.tile([P, 2, INNER], mmdt, tag="wk_sb")
            nc.scalar.dma_start(wk_sb[:], bc(wk.rearrange("(kt p) e -> p kt e", p=P)))
            wv_sb = pp.tile([P, 2, INNER], mmdt, tag="wv_sb")
            nc.gpsimd.dma_start(wv_sb[:], bc(wv.rearrange("(kt p) e -> p kt e", p=P)))
            wo_sb = pp.tile([P, 4, C], mmdt, tag="wo_sb")
            nc.gpsimd.dma_start(wo_sb[:], bc(wo.rearrange("(kt p) c -> p kt c", p=P)))
            ind_sb = pp.tile([P, P], mmdt, tag="ind_sb")
            nc.gpsimd.dma_start(ind_sb[:], bc(ind))
            g2_sb = pp.tile([P, 2], f32, tag="g2_sb")
            nc.gpsimd.dma_start(g2_sb[:], gamma2)
            b2_sb = pp.tile([P, 2], f32, tag="b2_sb")
            nc.gpsimd.dma_start(b2_sb[:], beta2)

            khat = pp.tile([P, 4, HEADS, D], mmdt, tag="khat")
            vt_sb = pp.tile([P, 4, INNER], mmdt, tag="vt_sb")
            qhat = pp.tile([P, 4, SL], mmdt, tag="qhat")
            o_sb = pp.tile([P, 4, SL], mmdt, tag="o_sb")
            s_sb = pp.tile([64, HEADS, D], f32, tag="s_sb")
            sfull_sb = pp.tile([P, HEADS, D], mmdt, tag="sfull_sb")
            ys_sb = pp.tile([P, 2, SL], f32, tag="ys_sb")
            yo_sb = pp.tile([P, 2, SL], f32, tag="yo_sb")
            stats_sb = pp.tile([P, 2, 2], f32, tag="stats_sb")
            stf_sb = pp.tile([P, 2, 2], f32, tag="stf_sb")

            # ---- stage KV + S partials (m = this core's 512 positions) ----
            # one PSUM bank per 128-col S region (matmul dst must be
            # bank-aligned; column-offset dst regions corrupt the bank)
            psum_S = [psS.tile([P, P], f32, tag=f"psum_S{pc}", name=f"psum_S{pc}")
                      for pc in range(4)]
            with tc.tile_pool(name="pskv", bufs=2, space="PSUM") as pskv:
                for mc in range(4):
                    xsl = xs_sb[:, :, mc * P:(mc + 1) * P]
                    psum_kT = pskv.tile([P, INNER], f32, tag="psum_kT")
                    psum_vT = pskv.tile([P, INNER], f32, tag="psum_vT")
                    for kt in range(2):
                        nc.tensor.matmul(psum_kT[:], mm(xsl[:, kt]), mm(wk_sb[:, kt]),
                                         start=(kt == 0), stop=(kt == 1))
                    for kt in range(2):
                        nc.tensor.matmul(psum_vT[:], mm(xsl[:, kt]), mm(wv_sb[:, kt]),
                                         start=(kt == 0), stop=(kt == 1))
                    # row norms of kT per head -> normalize
                    k2 = sp.tile([P, HEADS, D], f32, tag="k2")
                    nc.scalar.activation(k2[:], psum_kT.rearrange("p (g d) -> p g d", g=HEADS),
                                         AF.Square)
                    kn2 = sp.tile([P, HEADS], f32, tag="kn2")
                    nc.vector.tensor_reduce(kn2[:], k2[:], axis=mybir.AxisListType.X,
                                            op=ALU.add)
                    kn = sp.tile([P, HEADS], f32, tag="kn")
                    nc.scalar.activation(kn[:], kn2[:], AF.Sqrt, bias=SMOOTH)
                    kninv = sp.tile([P, HEADS], f32, tag="kninv")
                    nc.vector.reciprocal(kninv[:], kn[:])
                    nc.vector.tensor_tensor(
                        khat[:, mc], psum_kT.rearrange("p (g d) -> p g d", g=HEADS),
                        kninv[:, :, None].to_broadcast((P, HEADS, D)), ALU.mult)
                    nc.scalar.activation(vt_sb[:, mc], psum_vT[:], AF.Copy)
                    # S partial accumulation: 4 column regions of one bank
                    kh_flat = khat[:, mc].rearrange("p g d -> p (g d)")
                    for pc in range(4):
                        nc.tensor.matmul(
                            psum_S[pc][:],
                            mm(kh_flat[:, pc * P:(pc + 1) * P]),
                            mm(vt_sb[:, mc, pc * P:(pc + 1) * P]),
                            start=(mc == 0), stop=(mc == 3))

            for pc in range(4):
                nc.vector.tensor_copy(s_sb[:, 2 * pc, :], psum_S[pc][0:64, 0:64])
                nc.vector.tensor_copy(s_sb[:, 2 * pc + 1, :],
                                      psum_S[pc][64:128, 64:128])
            nc.sync.dma_start(s_part, s_sb.rearrange("p g d -> p (g d)"))
            if debug_out:
                nc.sync.dma_start(dbg["d_khat"], khat.bitcast(f32))
                nc.sync.dma_start(dbg["d_vt"], vt_sb.bitcast(f32))
                nc.sync.dma_start(dbg["d_spart"], s_sb[:])
            nc.gpsimd.collective_compute(
                "AllReduce", ALU.add, replica_groups=RG,
                ins=[s_part], outs=[s_full])
            sf_r = s_full.rearrange("p (g d) -> p g d", g=HEADS)
            nc.sync.dma_start(sfull_sb[0:64], bc(sf_r))
            nc.scalar.dma_start(sfull_sb[64:128], bc(sf_r))
            if debug_out:
                nc.sync.dma_start(dbg["d_sfull"], sfull_sb.bitcast(f32))

            # ---- stage Q (n = this core's 512 positions) ----
            with tc.tile_pool(name="psq", bufs=2, space="PSUM") as psq:
                for pc in range(4):
                    psum_q = psq.tile([P, SL], f32, tag="psum_q")
                    for kt in range(2):
                        nc.tensor.matmul(psum_q[:],
                                         mm(wq_sb[:, kt, pc * P:(pc + 1) * P]),
                                         mm(xs_sb[:, kt]),
                                         start=(kt == 0), stop=(kt == 1))
                    q2 = sp.tile([P, SL], mmdt, tag="q2")
                    nc.scalar.activation(q2[:], psum_q[:], AF.Square)
                    psum_nrm = psq.tile([P, SL], f32, tag="psum_nrm")
                    nc.tensor.matmul(psum_nrm[:], mm(ind_sb[:]), mm(q2[:]),
                                     start=True, stop=True)
                    qn = sp.tile([P, SL], f32, tag="qn")
                    nc.scalar.activation(qn[:], psum_nrm[:], AF.Sqrt, bias=SMOOTH)
                    qninv = sp.tile([P, SL], f32, tag="qninv")
                    nc.vector.reciprocal(qninv[:], qn[:])
                    nc.vector.tensor_tensor(qhat[:, pc], psum_q[:], qninv[:], ALU.mult)

            # ---- stage O: o[e,n] = sum_e' S[e',e] qhat[e',n], 2 heads/chunk ----
            with tc.tile_pool(name="pso", bufs=2, space="PSUM") as pso:
                for pc in range(4):
                    # two heads per chunk; PE row-groups 0-1 and 2-3 run
                    # these K=64 matmuls concurrently (row packing)
                    psum_oa = pso.tile([64, SL], f32, tag="psum_oa")
                    psum_ob = pso.tile([64, SL], f32, tag="psum_ob")
                    nc.tensor.matmul(psum_oa[:],
                                     mm(sfull_sb[0:64, 2 * pc, :]),
                                     mm(qhat[0:64, pc]),
                                     start=True, stop=True)
                    nc.tensor.matmul(psum_ob[:],
                                     mm(sfull_sb[64:128, 2 * pc + 1, :]),
                                     mm(qhat[64:128, pc]),
                                     start=True, stop=True)
                    nc.scalar.activation(o_sb[0:64, pc], psum_oa[:], AF.Copy)
                    nc.vector.tensor_copy(o_sb[64:128, pc], psum_ob[:])

            # ---- stage Y: y = w_out @ o, BN stats partials ----
            with tc.tile_pool(name="psy", bufs=2, space="PSUM") as psy:
                for cc in range(2):
                    psum_y = psy.tile([P, SL], f32, tag="psum_y")
                    for kt in range(4):
                        nc.tensor.matmul(psum_y[:],
                                         mm(wo_sb[:, kt, cc * P:(cc + 1) * P]),
                                         mm(o_sb[:, kt]),
                                         start=(kt == 0), stop=(kt == 3))
                    nc.scalar.activation(ys_sb[:, cc], psum_y[:], AF.Copy,
                                         accum_out=stats_sb[:, cc, 0:1])
                    junk = sp.tile([P, SL], f32, tag="junk")
                    nc.scalar.activation(junk[:], psum_y[:], AF.Square,
                                         accum_out=stats_sb[:, cc, 1:2])

            if debug_out:
                nc.sync.dma_start(dbg["d_qhat"], qhat.bitcast(f32))
                nc.sync.dma_start(dbg["d_o"], o_sb.bitcast(f32))
                nc.sync.dma_start(dbg["d_ys"], ys_sb[:])
                nc.sync.dma_start(dbg["d_stats"], stats_sb[:])
            nc.sync.dma_start(st_part, stats_sb.rearrange("p a b -> p (a b)"))
            nc.gpsimd.collective_compute(
                "AllReduce", ALU.add, replica_groups=RG,
                ins=[st_part], outs=[st_full])
            nc.sync.dma_start(stf_sb.rearrange("p a b -> p (a b)"), st_full)

            # ---- BN finalize: scale/shift per channel ----
            me = pp.tile([P, 2, 2], f32, tag="me")
            nc.vector.tensor_scalar_mul(me[:], stf_sb[:], 1.0 / N)
            mean = me[:, :, 0]
            ex2 = me[:, :, 1]
            var = pp.tile([P, 2], f32, tag="var")
            nc.vector.tensor_tensor(var[:], mean, mean, ALU.mult)
            nc.vector.tensor_tensor(var[:], ex2, var[:], ALU.subtract)
            std = pp.tile([P, 2], f32, tag="std")
            nc.scalar.activation(std[:], var[:], AF.Sqrt, bias=BN_EPS)
            rstd = pp.tile([P, 2], f32, tag="rstd")
            nc.vector.reciprocal(rstd[:], std[:])
            scale = pp.tile([P, 2], f32, tag="scale")
            nc.vector.tensor_tensor(scale[:], g2_sb[:], rstd[:], ALU.mult)
            shift = pp.tile([P, 2], f32, tag="shift")
            nc.vector.tensor_tensor(shift[:], mean[:], scale[:], ALU.mult)
            nc.vector.tensor_tensor(shift[:], b2_sb[:], shift[:], ALU.subtract)

            # ---- apply BN + ReLU, store ----
            y_r = y.rearrange("(cc p) n -> p cc n", p=P)
            for cc in range(2):
                nc.scalar.activation(yo_sb[:, cc], ys_sb[:, cc], AF.Relu,
                                     bias=shift[:, cc:cc + 1],
                                     scale=scale[:, cc:cc + 1])
                eng = nc.sync if cc == 0 else nc.scalar
                eng.dma_start(y_r[:, cc], yo_sb[:, cc])

    return nc


def _prep_inputs(x, w_qkv, w_out, gamma, beta):
    X = np.ascontiguousarray(x.reshape(C, N))
    wq = np.ascontiguousarray(w_qkv[0:INNER].T)
    wk = np.ascontiguousarray(w_qkv[INNER:2 * INNER].T)
    wv = np.ascontiguousarray(w_qkv[2 * INNER:3 * INNER].T)
    wo = np.ascontiguousarray(w_out.T)
    r = np.arange(P)
    ind = (((r[:, None] < D) == (r[None, :] < D))).astype(np.float32)
    ind = np.ascontiguousarray(ind)
    gamma2 = np.ascontiguousarray(gamma.reshape(2, P).T)
    beta2 = np.ascontiguousarray(beta.reshape(2, P).T)
    cvec = np.tile(np.array([SMOOTH, BN_EPS, 1.0 / N], np.float32), (P, 1))
    common = dict(wq=wq, wk=wk, wv=wv, wo=wo, ind=ind, gamma2=gamma2,
                  beta2=beta2, cvec=np.ascontiguousarray(cvec))
    in_maps = []
    for i in range(NCORES):
        m = dict(common)
        m["xs"] = np.ascontiguousarray(X[:, i * SL:(i + 1) * SL])
        in_maps.append(m)
    return in_maps


def kernel(x, w_qkv, w_out, gamma, beta):
    from concourse.bass_utils import run_bass_kernel_spmd

    if "nc" not in _CACHE:
        _CACHE["nc"] = build_nc()
    nc = _CACHE["nc"]

    in_maps = _prep_inputs(
        np.asarray(x, dtype=np.float32),
        np.asarray(w_qkv, dtype=np.float32),
        np.asarray(w_out, dtype=np.float32),
        np.asarray(gamma, dtype=np.float32),
        np.asarray(beta, dtype=np.float32),
    )
    res = run_bass_kernel_spmd(nc, in_maps, list(range(NCORES)))
    out = np.concatenate([res.results[i]["y"] for i in range(NCORES)], axis=1)
    return out.reshape(1, C, 64, 64).astype(np.float32)


def run_traced(x, w_qkv, w_out, gamma, beta, **kw):
    """Like kernel() but with NTFF tracing; returns (output, BassKernelResults)."""
    from concourse.bass_utils import run_bass_kernel_spmd

    if "nc" not in _CACHE:
        _CACHE["nc"] = build_nc()
    nc = _CACHE["nc"]
    in_maps = _prep_inputs(np.asarray(x, np.float32), np.asarray(w_qkv, np.float32),
                           np.asarray(w_out, np.float32), np.asarray(gamma, np.float32),
                           np.asarray(beta, np.float32))
    res = run_bass_kernel_spmd(nc, in_maps, list(range(NCORES)), trace=True, **kw)
    out = np.concatenate([res.results[i]["y"] for i in range(NCORES)], axis=1)
    return out.reshape(1, C, 64, 64).astype(np.float32), res
